# revision 9
# baseline (speedup 1.0000x reference)
"""Trainium2 Bass kernel for the coverage-attention module (sparse_attention).

Reference computation (B=64, S=1024, H2=1024):
    dec  = hidden @ W_feat + b_feat                       [B, H2]
    att  = feat.reshape(S,B,H2) + dec + cov*W_cov + b_cov [S, B, H2]
    sc   = tanh(att) @ W_att + b_att                      [S, B]
    a    = renorm(softmax(sc, axis=0) * mask)             [S, B]
    ctx  = einsum("sb,bsh->bh", a, encoder_out)           [B, H2]
    ncov = cov + a                                        [S, B]

Sharding: data-parallel over batch, 8 batches per core; softmax is over
seq (not sharded) so there are no collectives. Per-core layout puts h on
partitions and s on the free axis:
    att_T[h, s] = featT[h, s] + W_cov[h]*c[s] + (dec[b,h]+b_feat+b_cov)

The big streams (encoder_features, encoder_out, W_feat) are converted to
bf16 on the host (halves DMA; verified ~2e-3 output rel-err, 10x inside
the 2e-2 gate). DVE builds att (tensor_scalar w*c then add feat), ACT
applies tanh with the per-partition alpha bias, PE does the scores and
context dot products into f32 PSUM (bf16 matmuls hide their weight
loads in the background buffer; fp32 can't). Scores accumulate into one
[8, S] PSUM tile via zero-padded W_att columns because engine APs must
start at partition 0/32/64/96. Softmax runs in f32. b_att is dropped:
softmax is shift-invariant so it cannot affect outputs.
"""

import numpy as np
import ml_dtypes

import concourse.bass as bass
import concourse.mybir as mybir
import concourse.tile as tile
from concourse import bacc
from concourse.bass_utils import run_bass_kernel_spmd

N_CORES = 8
B, S, H = 64, 1024, 1024
BL = B // N_CORES          # batches per core
HB = H // 128              # h blocks
SBL = S // 128             # s blocks
F32 = mybir.dt.float32
BF16 = mybir.dt.bfloat16
HALF = 512
BF = ml_dtypes.bfloat16

FEAT_BUFS = 20
ENC_BUFS = 28


def build_nc():
    nc = bacc.Bacc("TRN2", debug=False, num_devices=N_CORES)

    featT = nc.dram_tensor("featT", [BL, H, S], BF16, kind="ExternalInput")
    enc = nc.dram_tensor("enc", [BL, S, H], BF16, kind="ExternalInput")
    hiddenT64 = nc.dram_tensor("hiddenT64", [128, 64], BF16, kind="ExternalInput")
    wfeat = nc.dram_tensor("wfeat", [H, H], BF16, kind="ExternalInput")
    wcov8 = nc.dram_tensor("wcov8", [128, HB], F32, kind="ExternalInput")
    watz = nc.dram_tensor("watz", [128, HB * BL * BL], BF16, kind="ExternalInput")
    bsum8 = nc.dram_tensor("bsum8", [128, HB], F32, kind="ExternalInput")
    covT = nc.dram_tensor("covT", [BL, S], F32, kind="ExternalInput")
    crows = nc.dram_tensor("crows", [1, BL * S], BF16, kind="ExternalInput")
    maskT = nc.dram_tensor("maskT", [BL, S], F32, kind="ExternalInput")
    ident = nc.dram_tensor("ident", [128, 128], F32, kind="ExternalInput")
    onesr = nc.dram_tensor("onesr", [1, 128], BF16, kind="ExternalInput")

    ctx_o = nc.dram_tensor("ctx_o", [BL, H], F32, kind="ExternalOutput")
    a_o = nc.dram_tensor("a_o", [BL, S], F32, kind="ExternalOutput")
    ncov_o = nc.dram_tensor("ncov_o", [BL, S], F32, kind="ExternalOutput")

    AF = mybir.ActivationFunctionType

    with tile.TileContext(nc) as tc:
        with tc.tile_pool(name="const", bufs=1) as const, \
             tc.tile_pool(name="wf", bufs=2) as wfp, \
             tc.tile_pool(name="feat", bufs=FEAT_BUFS) as fp, \
             tc.tile_pool(name="tanhp", bufs=3) as tp, \
             tc.tile_pool(name="tcovp", bufs=3) as tcp, \
             tc.tile_pool(name="cbcp", bufs=2) as cbp, \
             tc.tile_pool(name="crowp", bufs=2) as crp, \
             tc.tile_pool(name="ctxrowp", bufs=2) as cxp, \
             tc.tile_pool(name="encp", bufs=ENC_BUFS) as ep:

            idt = const.tile([128, 128], F32)
            nc.sync.dma_start(idt[:], ident.ap())
            onb = const.tile([1, 128], BF16)
            nc.sync.dma_start(onb[:], onesr.ap())
            hidt = const.tile([128, 64], BF16)
            nc.sync.dma_start(hidt[:], hiddenT64.ap())
            wc8 = const.tile([128, HB], F32)
            nc.sync.dma_start(wc8[:], wcov8.ap())
            wzt = const.tile([128, HB * BL * BL], BF16)
            nc.sync.dma_start(wzt[:], watz.ap())
            bst = const.tile([128, HB], F32)
            nc.sync.dma_start(bst[:], bsum8.ap())
            cvt = const.tile([BL, S], F32)
            nc.sync.dma_start(cvt[:], covT.ap())
            mkt = const.tile([BL, S], F32)
            nc.sync.dma_start(mkt[:], maskT.ap())

            # ---- dec = hidden @ W_feat (natural [b, h]) ----
            dec_sb = const.tile([BL, H], F32)
            with tc.tile_pool(name="pdec", bufs=1, space="PSUM") as pdp:
                pdec = pdp.tile([BL, H], F32)
                for kb in range(8):
                    wft = wfp.tile([128, H], BF16)
                    nc.sync.dma_start(wft[:], wfeat.ap()[kb * 128:(kb + 1) * 128, :])
                    for hf in range(2):
                        sl = slice(hf * HALF, (hf + 1) * HALF)
                        nc.tensor.matmul(
                            pdec[:, sl],
                            lhsT=hidt[:, kb * BL:(kb + 1) * BL],
                            rhs=wft[:, sl],
                            start=(kb == 0), stop=(kb == 7),
                        )
                nc.vector.tensor_copy(dec_sb[:], pdec[:])

            # ---- alpha[h, (hb,b)] = dec.T + (b_feat + b_cov) ----
            alpha = const.tile([128, HB * BL], F32)
            with tc.tile_pool(name="ptr", bufs=2, space="PSUM") as ptp:
                for hb in range(HB):
                    ptt = ptp.tile([128, BL], F32)
                    nc.tensor.transpose(
                        ptt[:], dec_sb[0:BL, hb * 128:(hb + 1) * 128], idt[0:BL, 0:BL]
                    )
                    nc.scalar.activation(
                        alpha[:, hb * BL:(hb + 1) * BL], ptt[:],
                        AF.Identity, bias=bst[:, hb:hb + 1],
                    )

            # ---- phase A: att -> tanh -> scores[8, S] in one PSUM tile ----
            scsb = const.tile([BL, S], F32)
            with tc.tile_pool(name="psc", bufs=1, space="PSUM") as psp, \
                 tc.tile_pool(name="pcb", bufs=2, space="PSUM") as pcbp:
                pscores = psp.tile([BL, S], F32)
                for b in range(BL):
                    # broadcast c row across partitions via rank-1 ones outer product
                    crt = crp.tile([1, S], BF16)
                    nc.sync.dma_start(crt[:], crows.ap()[0:1, b * S:(b + 1) * S])
                    pcb = pcbp.tile([128, S], F32)
                    for hf in range(2):
                        sl = slice(hf * HALF, (hf + 1) * HALF)
                        nc.tensor.matmul(pcb[:, sl], lhsT=onb[:], rhs=crt[0:1, sl],
                                         start=True, stop=True)
                    cbc = cbp.tile([128, S], BF16)
                    nc.scalar.copy(cbc[:], pcb[:])
                    for hb in range(HB):
                        ft = fp.tile([128, S], BF16)
                        nc.sync.dma_start(ft[:], featT.ap()[b, hb * 128:(hb + 1) * 128, :])
                        tcv = tcp.tile([128, S], BF16)
                        nc.vector.tensor_scalar_mul(tcv[:], cbc[:], wc8[:, hb:hb + 1])
                        att = tp.tile([128, S], BF16, tag="att")
                        nc.vector.tensor_add(att[:], ft[:], tcv[:])
                        th = tp.tile([128, S], BF16, tag="th")
                        nc.scalar.activation(
                            th[:], att[:], AF.Tanh,
                            bias=alpha[:, hb * BL + b: hb * BL + b + 1],
                        )
                        wslice = slice((hb * BL + b) * BL, (hb * BL + b + 1) * BL)
                        for hf in range(2):
                            sl = slice(hf * HALF, (hf + 1) * HALF)
                            nc.tensor.matmul(
                                pscores[:, sl], lhsT=wzt[:, wslice],
                                rhs=th[:, sl],
                                start=(b == 0 and hb == 0),
                                stop=(b == BL - 1 and hb == HB - 1),
                            )
                nc.vector.tensor_copy(scsb[:], pscores[:])

            # ---- phase B: masked softmax over s, renormalized (f32) ----
            neg = const.tile([BL, 1], F32)
            nc.vector.tensor_reduce(
                neg[:], scsb[:], axis=mybir.AxisListType.X,
                op=mybir.AluOpType.max, negate=True,
            )
            esb = const.tile([BL, S], F32)
            nc.scalar.activation(esb[:], scsb[:], AF.Exp, bias=neg[:, 0:1])
            nc.vector.tensor_mul(esb[:], esb[:], mkt[:])
            ssum = const.tile([BL, 1], F32)
            nc.vector.tensor_reduce(
                ssum[:], esb[:], axis=mybir.AxisListType.X, op=mybir.AluOpType.add
            )
            rs = const.tile([BL, 1], F32)
            nc.vector.reciprocal(rs[:], ssum[:])
            asb = const.tile([BL, S], F32)
            nc.vector.tensor_scalar_mul(asb[:], esb[:], rs[:, 0:1])
            ncv = const.tile([BL, S], F32)
            nc.vector.tensor_add(ncv[:], asb[:], cvt[:])
            nc.sync.dma_start(a_o.ap(), asb[:])
            nc.sync.dma_start(ncov_o.ap(), ncv[:])

            # a transposed to bf16 columns: aTb[s_in, (sb,b)]
            aTb = const.tile([128, SBL * BL], BF16)
            with tc.tile_pool(name="pta", bufs=2, space="PSUM") as ptap:
                for sb_i in range(SBL):
                    pta = ptap.tile([128, BL], F32)
                    nc.tensor.transpose(
                        pta[:], asb[0:BL, sb_i * 128:(sb_i + 1) * 128], idt[0:BL, 0:BL]
                    )
                    nc.scalar.copy(aTb[:, sb_i * BL:(sb_i + 1) * BL], pta[:])

            # ---- phase C: context = a @ encoder_out per batch ----
            with tc.tile_pool(name="pctx", bufs=2, space="PSUM") as pcp:
                for b in range(BL):
                    pc = pcp.tile([1, H], F32)
                    for sb_i in range(SBL):
                        et = ep.tile([128, H], BF16)
                        nc.sync.dma_start(et[:], enc.ap()[b, sb_i * 128:(sb_i + 1) * 128, :])
                        col = sb_i * BL + b
                        for hf in range(2):
                            sl = slice(hf * HALF, (hf + 1) * HALF)
                            nc.tensor.matmul(
                                pc[0:1, sl], lhsT=aTb[:, col:col + 1],
                                rhs=et[:, sl],
                                start=(sb_i == 0), stop=(sb_i == SBL - 1),
                            )
                    ctxrow = cxp.tile([1, H], F32)
                    nc.vector.tensor_copy(ctxrow[:], pc[:])
                    nc.sync.dma_start(ctx_o.ap()[b:b + 1, :], ctxrow[:])

    nc.compile()
    return nc


def make_in_maps(hidden, encoder_out, encoder_features, encoder_mask, coverage,
                 W_feat, b_feat, W_cov, b_cov, W_att, b_att):
    hidden = np.asarray(hidden, np.float32)
    encoder_out = np.asarray(encoder_out, np.float32)
    feat = np.asarray(encoder_features, np.float32).reshape(S, B, H)
    mask = np.asarray(encoder_mask, np.float32)
    cov = np.asarray(coverage, np.float32)
    W_feat16 = np.asarray(W_feat, np.float32).astype(BF)
    W_cov = np.asarray(W_cov, np.float32).reshape(H)
    W_att = np.asarray(W_att, np.float32).reshape(H)
    bsum = (np.asarray(b_feat, np.float32) + np.asarray(b_cov, np.float32))

    # zero-padded W_att columns: watz[k, ((hb*BL + b)*BL + j)] = W_att[hb*128+k] if j==b
    watz = np.zeros((128, HB * BL * BL), np.float32)
    for hb in range(HB):
        for b in range(BL):
            watz[:, (hb * BL + b) * BL + b] = W_att[hb * 128:(hb + 1) * 128]
    watz = watz.astype(BF)
    wcov8 = np.ascontiguousarray(W_cov.reshape(HB, 128).T)
    bsum8 = np.ascontiguousarray(bsum.reshape(HB, 128).T)
    identity = np.eye(128, dtype=np.float32)
    onesr = np.ones((1, 128), BF)

    in_maps = []
    for c in range(N_CORES):
        b0 = c * BL
        featT = np.ascontiguousarray(
            feat[:, b0:b0 + BL, :].transpose(1, 2, 0)).astype(BF)
        encl = np.ascontiguousarray(encoder_out[b0:b0 + BL]).astype(BF)
        hT = hidden[b0:b0 + BL].T                      # [H, BL]
        hiddenT64 = np.ascontiguousarray(
            hT.reshape(8, 128, BL).transpose(1, 0, 2).reshape(128, 64)).astype(BF)
        covT = np.ascontiguousarray(cov[:, b0:b0 + BL].T)
        maskTl = np.ascontiguousarray(mask[:, b0:b0 + BL].T)
        in_maps.append({
            "featT": featT,
            "enc": encl,
            "hiddenT64": hiddenT64,
            "wfeat": W_feat16,
            "wcov8": wcov8,
            "watz": watz,
            "bsum8": bsum8,
            "covT": covT,
            "crows": covT.reshape(1, BL * S).astype(BF),
            "maskT": maskTl,
            "ident": identity,
            "onesr": onesr,
        })
    return in_maps


_NC = None


def kernel(**inputs):
    global _NC
    if _NC is None:
        _NC = build_nc()
    in_maps = make_in_maps(**inputs)
    res = run_bass_kernel_spmd(_NC, in_maps, core_ids=list(range(N_CORES)))
    ctx = np.concatenate([res.results[c]["ctx_o"] for c in range(N_CORES)], axis=0)
    a_t = np.concatenate([res.results[c]["a_o"] for c in range(N_CORES)], axis=0)
    ncov_t = np.concatenate([res.results[c]["ncov_o"] for c in range(N_CORES)], axis=0)
    return (ctx.astype(np.float32),
            np.ascontiguousarray(a_t.T).astype(np.float32),
            np.ascontiguousarray(ncov_t.T).astype(np.float32))
